# revision 8
# baseline (speedup 1.0000x reference)
"""Chamfer loss kernel for 8 Trainium2 NeuronCores (Bass/Tile).

Problem: x [4,4096,3], y [4,4096,3] fp32 ->
  scalar = mean_m min_n ||x_n - y_m|| + mean_n min_m ||x_n - y_m||  (per batch, averaged)

Strategy
--------
s[m,n] = -||x_n - y_m||^2 = 2<y_m,x_n> - ||y_m||^2 - ||x_n||^2 = <yhat_m, xhat_n>
with yhat = [2y, -||y||^2, -1], xhat = [x, 1, ||x||^2]  (Da = 5).
Each side is split 2-way into bf16 (h1+h2); the products (h1 g1, h1 g2, h2 g1)
are stacked along the matmul contraction axis (the h2 g2 block is ~2^-18 and
dropped) -> K = 15 <= 128, so the PE computes fp32-grade s tiles at bf16
speed (cost ~ N columns).

Both chamfer directions are max-reductions of s (sqrt is monotonic, applied on
host to the reduced values only):
  dist1[m] = max_n s[m,n]   (free axis  -> fp16 fold chain + one batched reduce)
  dist2[n] = max_m s[m,n]   (partition/tile axis -> fp16 tensor_max chain into
                             acc2; the 128-way partition max finishes on host)

Engine balance (measured): the wall is the ScalarE (ACT) PSUM->SBUF
evacuation (65536 fp32/lane/rep at ~1 elem/cycle/lane @1.2GHz ~= 57-59us);
the DVE max-chains run just under it and the PE (2 bf16 cols/cycle) far
under (~12-25us). The PSUM layout ([128,2048] x 2 banks ping-pong, one ACT
copy per 2048-chunk) keeps ACT busy back-to-back, which is what matters.

Sharding: core c -> batch b = c//2, m-half h = c%2 (2048 m x 4096 n per core).
dist1 is exact per core; dist2 partials are max-combined on host.
"""

import sys

if "/opt/trn_rl_repo" not in sys.path:
    sys.path.insert(0, "/opt/trn_rl_repo")

from contextlib import ExitStack

import numpy as np
import ml_dtypes

import concourse.bass as bass
import concourse.tile as tile
from concourse import bacc, mybir
from concourse.bass_utils import run_bass_kernel_spmd

B, N, M, D = 4, 4096, 4096, 3
DA = 5          # augmented vector length
KS = 3 * DA     # 15: (h1g1, h1g2, h2g1) block products stacked on contraction axis
MT = 16         # m-tiles of 128 per core (2048 m's)
NG = 2          # psum groups of 4 n-chunks (2048 n's each)
GW = 4 * 512    # group width
P = 128

FP32 = mybir.dt.float32
FP16 = mybir.dt.float16
BF16 = mybir.dt.bfloat16


def build_program(repeat: int = 1, mt: int = MT, offload: int = 0,
                  hoist: bool = True, probe=()):
    """Build the SPMD bass program. Returns compiled Bacc object.

    offload: number of 2048-wide PSUM evacuations per rep (out of 2*mt)
    moved from the ACT engine to the DVE (tensor_copy at 1x). Measured
    neutral-to-worse (DVE is saturated) — keep 0.

    hoist: allocate the psum/c/junk tile pools once for all reps instead of
    per rep. Removes per-rep pool release/realloc overhead (~10us/rep in the
    CoreSim model, a few us on HW); verified race-free (CoreSim) and correct
    on HW at repeat>1.
    """
    nc = bacc.Bacc("TRN2", target_bir_lowering=False, debug=False, num_devices=8)

    ys_d = nc.dram_tensor("ys", [KS, MT * P], BF16, kind="ExternalInput").ap()
    xs_d = nc.dram_tensor("xs", [KS, N], BF16, kind="ExternalInput").ap()
    out_d = nc.dram_tensor("out", [P, MT], FP32, kind="ExternalOutput").ap()
    acc_d = nc.dram_tensor("acc", [P, N], FP16, kind="ExternalOutput").ap()

    # offloaded (engine=DVE) evacuations: spread over the rep, g=1 groups only
    off_set = set()
    if offload:
        stride = max(1, mt // max(offload, 1))
        for i in range(offload):
            off_set.add(min(mt - 1, (i + 1) * stride - 1))

    with tile.TileContext(nc) as tc, ExitStack() as ctx:
        consts = ctx.enter_context(tc.tile_pool(name="consts", bufs=1))
        y_sb = consts.tile([KS, MT * P], BF16, tag="y_sb")
        x_sb = consts.tile([KS, N], BF16, tag="x_sb")
        nc.sync.dma_start(y_sb[:], ys_d[:])
        nc.sync.dma_start(x_sb[:], xs_d[:])

        d1 = consts.tile([P, MT], FP32, tag="d1")       # dist1: col t
        # folded dist1 rows: col block t holds 512-wide folded maxima
        w1 = consts.tile([P, MT * 512], FP16, tag="w1")
        # dist2 accumulator: col f = n; partition-axis max finished on host
        acc2 = consts.tile([P, N], FP16, tag="acc2")
        cfix = None
        if "nocopy" in probe:
            cfix = consts.tile([P, N], FP16, tag="cfix")
            nc.vector.memset(cfix[:], 0.0)
        if "nochain" in probe:
            nc.vector.memset(acc2[:], 0.0)
        if "nofold" in probe:
            nc.vector.memset(w1[:], 0.0)

        hoisted = None
        if hoist or "hoist" in probe:
            hoisted = (
                ctx.enter_context(tc.tile_pool(name="psum", bufs=2, space="PSUM")),
                ctx.enter_context(tc.tile_pool(name="cpool", bufs=4)),
                ctx.enter_context(tc.tile_pool(name="junk", bufs=2)),
            )

        for _rep in range(repeat):
            with ExitStack() as rep_ctx:
                if hoisted is not None:
                    psum_pool, cpool, junkpool = hoisted
                else:
                    psum_pool = rep_ctx.enter_context(
                        tc.tile_pool(name="psum", bufs=2, space="PSUM"))
                    cpool = rep_ctx.enter_context(tc.tile_pool(name="cpool", bufs=4))
                    junkpool = rep_ctx.enter_context(tc.tile_pool(name="junk", bufs=2))
                for t in range(mt):
                    lhs = y_sb[:, t * P:(t + 1) * P]
                    c = cfix if cfix is not None else cpool.tile([P, N], FP16)
                    for g in range(NG):
                        ps = psum_pool.tile([P, GW], FP32)
                        for k in range(4):
                            nc.tensor.matmul(
                                ps[:, k * 512:(k + 1) * 512],
                                lhsT=lhs,
                                rhs=x_sb[:, (g * 4 + k) * 512:(g * 4 + k + 1) * 512],
                                start=True,
                                stop=True,
                            )
                        if "nocopy" not in probe:
                            if g == 1 and t in off_set:
                                nc.vector.tensor_copy(c[:, g * GW:(g + 1) * GW], ps[:])
                            else:
                                nc.scalar.copy(c[:, g * GW:(g + 1) * GW], ps[:])
                    # dist2 chain: one wide fp16 op over all 4096 n's
                    if "nochain" in probe:
                        pass
                    elif _rep == 0 and t == 0:
                        nc.vector.tensor_copy(acc2[:], c[:])
                    else:
                        nc.vector.tensor_max(acc2[:], acc2[:], c[:])
                    # dist1 funnel: fp16 fold chain (2x_1p) then one batched reduce.
                    if "nofold" in probe:
                        continue
                    jk = junkpool.tile([P, GW], FP16)
                    nc.vector.tensor_max(jk[:], c[:, 0:GW], c[:, GW:N])
                    nc.vector.tensor_max(
                        jk[:, 0:1024], jk[:, 0:1024], jk[:, 1024:2048]
                    )
                    nc.vector.tensor_max(
                        w1[:, t * 512:(t + 1) * 512], jk[:, 0:512], jk[:, 512:1024]
                    )
                # one batched reduce for all m-tiles: [P, mt, 512] -> [P, mt]
                nc.vector.tensor_reduce(
                    d1[:, 0:mt],
                    w1[:, 0:mt * 512].rearrange("p (t q) -> p t q", t=mt),
                    axis=mybir.AxisListType.X,
                    op=mybir.AluOpType.max,
                )

        # dist2 partition-axis max is finished on host: ship acc2 as-is.
        nc.sync.dma_start(out_d[:], d1[:])
        nc.sync.dma_start(acc_d[:], acc2[:])

    nc.compile()
    return nc


def _np2split(v: np.ndarray):
    """2-way bf16 split of float64 array v: returns (h1,h2) bf16."""
    v = v.astype(np.float64)
    h1 = v.astype(ml_dtypes.bfloat16)
    r1 = v - h1.astype(np.float64)
    h2 = r1.astype(ml_dtypes.bfloat16)
    return h1, h2


def make_inputs(x: np.ndarray, y: np.ndarray):
    """Host prep: augmented, 2-way-split, K-stacked operands per core."""
    x = np.asarray(x, dtype=np.float32)
    y = np.asarray(y, dtype=np.float32)
    x64 = x.astype(np.float64)
    y64 = y.astype(np.float64)
    x2 = (x64 * x64).sum(-1)  # [B,N]
    y2 = (y64 * y64).sum(-1)  # [B,M]

    # xhat [B,DA,N], yhat [B,DA,M]
    xhat = np.empty((B, DA, N), np.float64)
    xhat[:, 0:3, :] = x64.transpose(0, 2, 1)
    xhat[:, 3, :] = 1.0
    xhat[:, 4, :] = x2
    yhat = np.empty((B, DA, M), np.float64)
    yhat[:, 0:3, :] = 2.0 * y64.transpose(0, 2, 1)
    yhat[:, 3, :] = -y2
    yhat[:, 4, :] = -1.0

    xh = _np2split(xhat)  # each [B,DA,N] bf16
    yh = _np2split(yhat)

    # K-stack: blocks (h1 g1, h1 g2, h2 g1); h2 g2 ~ 2^-18 relative, dropped
    blocks = [(0, 0), (0, 1), (1, 0)]
    xs = np.empty((B, KS, N), ml_dtypes.bfloat16)
    ys = np.empty((B, KS, M), ml_dtypes.bfloat16)
    for blk, (i, j) in enumerate(blocks):
        ys[:, blk * DA:(blk + 1) * DA, :] = yh[i]
        xs[:, blk * DA:(blk + 1) * DA, :] = xh[j]

    in_maps = []
    for c in range(8):
        b, h = c // 2, c % 2
        in_maps.append({
            "ys": np.ascontiguousarray(ys[b, :, h * 2048:(h + 1) * 2048]),
            "xs": np.ascontiguousarray(xs[b]),
        })
    return in_maps


def combine(results):
    """Host combine: per core "out" [128,16] fp32, "acc" [128,4096] fp16."""
    smax1 = np.empty((B, M), np.float64)  # max_n s  (dist1 dir)
    smax2 = np.full((B, N), -np.inf, np.float64)  # max_m s (dist2 dir)
    for c in range(8):
        b, h = c // 2, c % 2
        d1 = np.asarray(results[c]["out"], np.float64)  # [128,16]: col t, m=h*2048+t*128+p
        smax1[b, h * 2048:(h + 1) * 2048] = d1.T.reshape(-1)
        acc = np.asarray(results[c]["acc"]).astype(np.float64)  # [128, 4096]: col = n
        smax2[b] = np.maximum(smax2[b], acc.max(axis=0))
    d2min_m = np.maximum(-smax1, 0.0)
    d2min_n = np.maximum(-smax2, 0.0)
    loss = np.sqrt(d2min_m).mean() + np.sqrt(d2min_n).mean()
    return np.float32(loss)


_CACHE = {}


def kernel(x, y):
    if "nc" not in _CACHE:
        _CACHE["nc"] = build_program(repeat=1)
    nc = _CACHE["nc"]
    in_maps = make_inputs(x, y)
    res = run_bass_kernel_spmd(nc, in_maps, list(range(8)))
    return combine(res.results)


# revision 11
# speedup vs baseline: 1.1499x; 1.1499x over previous
"""Chamfer loss kernel for 8 Trainium2 NeuronCores (Bass/Tile).

Problem: x [4,4096,3], y [4,4096,3] fp32 ->
  scalar = mean_m min_n ||x_n - y_m|| + mean_n min_m ||x_n - y_m||  (per batch, averaged)

Strategy
--------
s[m,n] = -||x_n - y_m||^2 = 2<y_m,x_n> - ||y_m||^2 - ||x_n||^2 = <yhat_m, xhat_n>
with yhat = [2y, -||y||^2, -1], xhat = [x, 1, ||x||^2]  (Da = 5).
Each side is split 2-way into bf16 (h1+h2); the products (h1 g1, h1 g2, h2 g1)
are stacked along the matmul contraction axis (the h2 g2 block is ~2^-18 and
dropped) -> K = 15 <= 128, so the PE computes fp32-grade s tiles at bf16
speed (cost ~ N columns).

Both chamfer directions are max-reductions of s (sqrt is monotonic, applied on
host to the reduced values only):
  dist1[m] = max_n s[m,n]   (free axis  -> fp16 fold chain + one batched reduce)
  dist2[n] = max_m s[m,n]   (partition/tile axis -> fp16 tensor_max chain into
                             acc2; the 128-way partition max finishes on host)

Engine balance (measured): the wall is the ScalarE (ACT) PSUM->SBUF
evacuation (65536 fp32/lane/rep at ~1 elem/cycle/lane @1.2GHz ~= 57-59us);
the DVE max-chains run just under it and the PE (2 bf16 cols/cycle) far
under (~12-25us). The PSUM layout ([128,2048] x 2 banks ping-pong, one ACT
copy per 2048-chunk) keeps ACT busy back-to-back, which is what matters.

Sharding: core c -> batch b = c//2, m-half h = c%2 (2048 m x 4096 n per core).
dist1 is exact per core; dist2 partials are max-combined on host.
"""

import sys

if "/opt/trn_rl_repo" not in sys.path:
    sys.path.insert(0, "/opt/trn_rl_repo")

from contextlib import ExitStack

import numpy as np
import ml_dtypes

import concourse.bass as bass
import concourse.tile as tile
from concourse import bacc, mybir
from concourse.bass_utils import run_bass_kernel_spmd

B, N, M, D = 4, 4096, 4096, 3
DA = 5          # augmented vector length
KS = 3 * DA     # 15: (h1g1, h1g2, h2g1) block products stacked on contraction axis
MT = 16         # m-tiles of 128 per core (2048 m's)
NG = 2          # psum groups of 4 n-chunks (2048 n's each)
GW = 4 * 512    # group width
P = 128

FP32 = mybir.dt.float32
FP16 = mybir.dt.float16
BF16 = mybir.dt.bfloat16


def build_program(repeat: int = 1, mt: int = MT, offload: int = 0,
                  hoist: bool = True, probe=()):
    """Build the SPMD bass program. Returns compiled Bacc object.

    offload: number of 2048-wide PSUM evacuations per rep (out of 2*mt)
    moved from the ACT engine to the DVE (tensor_copy at 1x). Measured
    neutral-to-worse (DVE is saturated) — keep 0.

    hoist: allocate the psum/c/junk tile pools once for all reps instead of
    per rep. Removes per-rep pool release/realloc overhead (~10us/rep in the
    CoreSim model, a few us on HW); verified race-free (CoreSim) and correct
    on HW at repeat>1.
    """
    nc = bacc.Bacc("TRN2", target_bir_lowering=False, debug=False, num_devices=8)

    ys_d = nc.dram_tensor("ys", [KS, MT * P], BF16, kind="ExternalInput").ap()
    xs_d = nc.dram_tensor("xs", [KS, N], BF16, kind="ExternalInput").ap()
    out_d = nc.dram_tensor("out", [P, MT], FP32, kind="ExternalOutput").ap()
    acc_d = nc.dram_tensor("acc", [P, N], FP16, kind="ExternalOutput").ap()

    # offloaded (engine=DVE) evacuations: spread over the rep, g=1 groups only
    off_set = set()
    if offload:
        stride = max(1, mt // max(offload, 1))
        for i in range(offload):
            off_set.add(min(mt - 1, (i + 1) * stride - 1))

    with tile.TileContext(nc) as tc, ExitStack() as ctx:
        consts = ctx.enter_context(tc.tile_pool(name="consts", bufs=1))
        y_sb = consts.tile([KS, MT * P], BF16, tag="y_sb")
        x_sb = consts.tile([KS, N], BF16, tag="x_sb")
        nc.sync.dma_start(y_sb[:], ys_d[:])
        nc.sync.dma_start(x_sb[:], xs_d[:])

        # 3-level fold funnel is the default: tensor_reduce runs at 1x on HW
        # (CoreSim model agrees; fold1 A/B measured worse), so minimizing the
        # batched-reduce volume beats minimizing DVE op count.
        fold1 = "fold1" in probe
        d1 = consts.tile([P, MT], FP32, tag="d1")       # dist1: col t
        # folded dist1 rows: col block t holds the folded maxima
        w1 = consts.tile([P, MT * (2048 if fold1 else 512)], FP16, tag="w1")
        # dist2 accumulator: col f = n; partition-axis max finished on host
        acc2 = consts.tile([P, N], FP16, tag="acc2")
        cfix = None
        if "nocopy" in probe:
            cfix = consts.tile([P, N], FP16, tag="cfix")
            nc.vector.memset(cfix[:], 0.0)
        if "nochain" in probe:
            nc.vector.memset(acc2[:], 0.0)
        if "nofold" in probe:
            nc.vector.memset(w1[:], 0.0)

        hoisted = None
        if hoist or "hoist" in probe:
            hoisted = (
                ctx.enter_context(tc.tile_pool(name="psum", bufs=2, space="PSUM")),
                ctx.enter_context(tc.tile_pool(name="cpool", bufs=4)),
                ctx.enter_context(tc.tile_pool(name="junk", bufs=2)),
            )

        for _rep in range(repeat):
            with ExitStack() as rep_ctx:
                if hoisted is not None:
                    psum_pool, cpool, junkpool = hoisted
                else:
                    psum_pool = rep_ctx.enter_context(
                        tc.tile_pool(name="psum", bufs=2, space="PSUM"))
                    cpool = rep_ctx.enter_context(tc.tile_pool(name="cpool", bufs=4))
                    junkpool = rep_ctx.enter_context(tc.tile_pool(name="junk", bufs=2))
                for t in range(mt):
                    lhs = y_sb[:, t * P:(t + 1) * P]
                    c = cfix if cfix is not None else cpool.tile([P, N], FP16)
                    for g in range(NG):
                        ps = psum_pool.tile([P, GW], FP32)
                        for k in range(4):
                            nc.tensor.matmul(
                                ps[:, k * 512:(k + 1) * 512],
                                lhsT=lhs,
                                rhs=x_sb[:, (g * 4 + k) * 512:(g * 4 + k + 1) * 512],
                                start=True,
                                stop=True,
                            )
                        if "nocopy" not in probe:
                            if g == 1 and t in off_set:
                                nc.vector.tensor_copy(c[:, g * GW:(g + 1) * GW], ps[:])
                            else:
                                nc.scalar.copy(c[:, g * GW:(g + 1) * GW], ps[:])
                    # dist2 chain: one wide fp16 op over all 4096 n's
                    if "nochain" in probe:
                        pass
                    elif _rep == 0 and t == 0:
                        nc.vector.tensor_copy(acc2[:], c[:])
                    else:
                        nc.vector.tensor_max(acc2[:], acc2[:], c[:])
                    # dist1 funnel: fp16 fold (2x_1p) then one batched reduce.
                    if "nofold" in probe:
                        continue
                    if fold1:
                        # single fold level: one DVE op per tile. Same total
                        # DVE cycles as the 3-level funnel (read-bandwidth
                        # conserved) but half the DVE op count -> fewer
                        # per-op overheads and pipe drains.
                        nc.vector.tensor_max(
                            w1[:, t * GW:(t + 1) * GW], c[:, 0:GW], c[:, GW:N]
                        )
                    else:
                        jk = junkpool.tile([P, GW], FP16)
                        nc.vector.tensor_max(jk[:], c[:, 0:GW], c[:, GW:N])
                        nc.vector.tensor_max(
                            jk[:, 0:1024], jk[:, 0:1024], jk[:, 1024:2048]
                        )
                        nc.vector.tensor_max(
                            w1[:, t * 512:(t + 1) * 512], jk[:, 0:512], jk[:, 512:1024]
                        )
                # one batched reduce for all m-tiles: [P, mt, q] -> [P, mt]
                qw = GW if fold1 else 512
                nc.vector.tensor_reduce(
                    d1[:, 0:mt],
                    w1[:, 0:mt * qw].rearrange("p (t q) -> p t q", t=mt),
                    axis=mybir.AxisListType.X,
                    op=mybir.AluOpType.max,
                )

        # dist2 partition-axis max is finished on host: ship acc2 as-is.
        nc.sync.dma_start(out_d[:], d1[:])
        nc.sync.dma_start(acc_d[:], acc2[:])

    nc.compile()
    return nc


def _np2split(v: np.ndarray):
    """2-way bf16 split of float64 array v: returns (h1,h2) bf16."""
    v = v.astype(np.float64)
    h1 = v.astype(ml_dtypes.bfloat16)
    r1 = v - h1.astype(np.float64)
    h2 = r1.astype(ml_dtypes.bfloat16)
    return h1, h2


def make_inputs(x: np.ndarray, y: np.ndarray):
    """Host prep: augmented, 2-way-split, K-stacked operands per core."""
    x = np.asarray(x, dtype=np.float32)
    y = np.asarray(y, dtype=np.float32)
    x64 = x.astype(np.float64)
    y64 = y.astype(np.float64)
    x2 = (x64 * x64).sum(-1)  # [B,N]
    y2 = (y64 * y64).sum(-1)  # [B,M]

    # xhat [B,DA,N], yhat [B,DA,M]
    xhat = np.empty((B, DA, N), np.float64)
    xhat[:, 0:3, :] = x64.transpose(0, 2, 1)
    xhat[:, 3, :] = 1.0
    xhat[:, 4, :] = x2
    yhat = np.empty((B, DA, M), np.float64)
    yhat[:, 0:3, :] = 2.0 * y64.transpose(0, 2, 1)
    yhat[:, 3, :] = -y2
    yhat[:, 4, :] = -1.0

    xh = _np2split(xhat)  # each [B,DA,N] bf16
    yh = _np2split(yhat)

    # K-stack: blocks (h1 g1, h1 g2, h2 g1); h2 g2 ~ 2^-18 relative, dropped
    blocks = [(0, 0), (0, 1), (1, 0)]
    xs = np.empty((B, KS, N), ml_dtypes.bfloat16)
    ys = np.empty((B, KS, M), ml_dtypes.bfloat16)
    for blk, (i, j) in enumerate(blocks):
        ys[:, blk * DA:(blk + 1) * DA, :] = yh[i]
        xs[:, blk * DA:(blk + 1) * DA, :] = xh[j]

    in_maps = []
    for c in range(8):
        b, h = c // 2, c % 2
        in_maps.append({
            "ys": np.ascontiguousarray(ys[b, :, h * 2048:(h + 1) * 2048]),
            "xs": np.ascontiguousarray(xs[b]),
        })
    return in_maps


def combine(results):
    """Host combine: per core "out" [128,16] fp32, "acc" [128,4096] fp16."""
    smax1 = np.empty((B, M), np.float64)  # max_n s  (dist1 dir)
    smax2 = np.full((B, N), -np.inf, np.float64)  # max_m s (dist2 dir)
    for c in range(8):
        b, h = c // 2, c % 2
        d1 = np.asarray(results[c]["out"], np.float64)  # [128,16]: col t, m=h*2048+t*128+p
        smax1[b, h * 2048:(h + 1) * 2048] = d1.T.reshape(-1)
        acc = np.asarray(results[c]["acc"]).astype(np.float64)  # [128, 4096]: col = n
        smax2[b] = np.maximum(smax2[b], acc.max(axis=0))
    d2min_m = np.maximum(-smax1, 0.0)
    d2min_n = np.maximum(-smax2, 0.0)
    loss = np.sqrt(d2min_m).mean() + np.sqrt(d2min_n).mean()
    return np.float32(loss)


_CACHE = {}


def kernel(x, y):
    if "nc" not in _CACHE:
        _CACHE["nc"] = build_program(repeat=1)
    nc = _CACHE["nc"]
    in_maps = make_inputs(x, y)
    res = run_bass_kernel_spmd(nc, in_maps, list(range(8)))
    return combine(res.results)
